# Initial kernel scaffold
#
"""Trainium2 Bass kernel for DirectG2PNEFS: 3-layer biLSTM encoder (B=256,T=512,H=256)
+ 100-step argmax-feedback LSTM decoder.

Sharding: 8 cores = 2 directions x 4 batch-quarters (64 rows each). One SPMD
program; all divergence is data-driven. Matmuls in bf16 (validated: 0 argmax
flips, ~4e-3 rel err vs f32), cell state bf16, PSUM accumulation f32.
"""
import numpy as np
import ml_dtypes

import concourse.bass as bass
import concourse.mybir as mybir
import concourse.tile as tile
from concourse.masks import make_identity
from concourse import bass_isa

BF16 = mybir.dt.bfloat16
F32 = mybir.dt.float32
I32 = mybir.dt.int32
AF = mybir.ActivationFunctionType
nbf16 = ml_dtypes.bfloat16

H = 256
V = 256
G = 4 * H  # 1024 gates
# gate reorder: torch order i,f,g,o -> ours i,f,o,g (so sigmoid group is contiguous)
GATE_PERM = np.concatenate([np.arange(0, H), np.arange(H, 2 * H),
                            np.arange(3 * H, 4 * H), np.arange(2 * H, 3 * H)])


def _reord(w, axis):
    return np.take(w, GATE_PERM, axis=axis)


def build_program(B_loc, T, DEC, CHUNK):
    """Build the SPMD bass program. B_loc=64 rows/core, T encoder steps,
    DEC decoder steps, CHUNK = encoder steps per input-staging chunk."""
    assert T % CHUNK == 0
    NCH = T // CHUNK
    nc = bass.Bass()

    # ---------------- I/O ----------------
    textT = nc.declare_dram_parameter("textT", [1, T * B_loc], I32)
    tbl0 = nc.declare_dram_parameter("tbl0", [2, 128, G], BF16)
    whh0 = nc.declare_dram_parameter("whh0", [2, 128, G], BF16)
    wih1 = nc.declare_dram_parameter("wih1", [4, 128, G], BF16)
    whh1 = nc.declare_dram_parameter("whh1", [2, 128, G], BF16)
    bias1 = nc.declare_dram_parameter("bias1", [1, G], BF16)
    wih2 = nc.declare_dram_parameter("wih2", [4, 128, G], BF16)
    whh2 = nc.declare_dram_parameter("whh2", [2, 128, G], BF16)
    bias2 = nc.declare_dram_parameter("bias2", [1, G], BF16)
    tbld = nc.declare_dram_parameter("tbld", [2, 128, G], BF16)
    dwih0f = nc.declare_dram_parameter("dwih0f", [4, 128, G], BF16)  # full 512-in, step 0
    dbias0 = nc.declare_dram_parameter("dbias0", [1, G], BF16)
    dwih = nc.declare_dram_parameter("dwih", [2, 2, 128, G], BF16)  # dec layers 1,2
    dwhh = nc.declare_dram_parameter("dwhh", [3, 2, 128, G], BF16)
    dbias = nc.declare_dram_parameter("dbias", [2, 1, G], BF16)
    projwT = nc.declare_dram_parameter("projwT", [2, 128, V], BF16)
    projb = nc.declare_dram_parameter("projb", [128, 2], F32)
    logitsT_out = nc.declare_dram_parameter("logitsT", [2, 128, DEC * B_loc], BF16,
                                            isOutput=True)

    # internal DRAM
    send = nc.dram_tensor("send", [2, 128, T * B_loc], BF16)
    recv = nc.dram_tensor("recv", [2, 2, 128, T * B_loc], BF16, addr_space="Shared")
    GROUPS = [[0, 4], [1, 5], [2, 6], [3, 7]]

    with TileKernel(nc) as tc:
        _body(nc, tc, locals(), B_loc, T, DEC, CHUNK, NCH)
    return nc


class TileKernel:
    """Thin wrapper so build stays flat."""
    def __init__(self, nc):
        self.tc = tile.TileContext(nc)

    def __enter__(self):
        return self.tc.__enter__()

    def __exit__(self, *a):
        return self.tc.__exit__(*a)


def _cell(nc, pools, gates_ps, c_prev, B_loc):
    """LSTM cell from PSUM gates (B_loc, 1024) f32 -> h (B_loc,256) bf16, c bf16."""
    sb = pools["cell"]
    sig = sb.tile([B_loc, 3 * H], BF16, tag="sig")
    nc.scalar.activation(sig[:], gates_ps[:, 0:3 * H], AF.Sigmoid)
    gt = sb.tile([B_loc, H], BF16, tag="gt")
    nc.scalar.activation(gt[:], gates_ps[:, 3 * H:4 * H], AF.Tanh)
    w = sb.tile([B_loc, H], BF16, tag="w")
    nc.vector.tensor_mul(w[:], sig[:, 0:H], gt[:])
    t1 = sb.tile([B_loc, H], BF16, tag="t1")
    nc.vector.tensor_mul(t1[:], sig[:, H:2 * H], c_prev[:])
    c_new = sb.tile([B_loc, H], BF16, tag="c")
    nc.vector.tensor_add(c_new[:], t1[:], w[:])
    tc_t = sb.tile([B_loc, H], BF16, tag="tc")
    nc.scalar.activation(tc_t[:], c_new[:], AF.Tanh)
    h = sb.tile([B_loc, H], BF16, tag="h")
    nc.vector.tensor_mul(h[:], sig[:, 2 * H:3 * H], tc_t[:])
    return h, c_new


def _transpose_h(nc, pools, h, identity, B_loc, tag="hT"):
    """h (B_loc,256) bf16 -> two SBUF tiles (128, B_loc) bf16 (transposed)."""
    pps = pools["psumT"]
    sb = pools["hT"]
    outs = []
    for k in range(2):
        pt = pps.tile([128, B_loc], BF16, tag=f"{tag}ps{k}")
        nc.tensor.transpose(pt[:], h[:, k * 128:(k + 1) * 128], identity[:B_loc, :B_loc])
        st = sb.tile([128, B_loc], BF16, tag=f"{tag}sb{k}")
        nc.vector.tensor_copy(st[:], pt[:])
        outs.append(st)
    return outs


def _body(nc, tc, env, B_loc, T, DEC, CHUNK, NCH):
    textT, tbl0, whh0 = env["textT"], env["tbl0"], env["whh0"]
    wih1, whh1, bias1 = env["wih1"], env["whh1"], env["bias1"]
    wih2, whh2, bias2 = env["wih2"], env["whh2"], env["bias2"]
    tbld, dwih0f, dbias0 = env["tbld"], env["dwih0f"], env["dbias0"]
    dwih, dwhh, dbias = env["dwih"], env["dwhh"], env["dbias"]
    projwT, projb, logitsT_out = env["projwT"], env["projb"], env["logitsT"]
    send, recv, GROUPS = env["send"], env["recv"], env["GROUPS"]

    ctxpools = {}
    consts = tc.tile_pool(name="consts", bufs=1).__enter__()
    wpool = tc.tile_pool(name="weights", bufs=1).__enter__()
    xin = tc.tile_pool(name="xin", bufs=2).__enter__()
    cellp = tc.tile_pool(name="cellp", bufs=2).__enter__()
    hTp = tc.tile_pool(name="hTp", bufs=2).__enter__()
    psg = tc.tile_pool(name="psg", bufs=2, space="PSUM").__enter__()
    psT = tc.tile_pool(name="psT", bufs=2, space="PSUM").__enter__()
    pools = {"cell": cellp, "psumT": psT, "hT": hTp}

    # constants
    ident = consts.tile([128, 128], BF16)
    make_identity(nc, ident[:])
    ones1 = consts.tile([1, B_loc], BF16)
    nc.gpsimd.memset(ones1[:], 1.0)
    iota2 = []
    for k in range(2):
        it = consts.tile([128, CHUNK * B_loc], I32, tag=f"iota{k}")
        nc.gpsimd.iota(it[:], pattern=[[0, CHUNK * B_loc]], base=k * 128,
                       channel_multiplier=1)
        iota2.append(it)

    projb_sb = consts.tile([128, 2], F32)
    nc.sync.dma_start(projb_sb[:], projb[:])

    # ---------------- encoder ----------------
    for layer in range(3):
        # load this layer's weights
        if layer == 0:
            rhs_x = wpool.tile([2, 128, G], BF16, tag="rhsx")
            nc.sync.dma_start(rhs_x[:], tbl0[:])
            whh_l, bias_l, nkx = whh0, None, 2
        else:
            rhs_x = wpool.tile([4, 128, G], BF16, tag="rhsx4")
            nc.sync.dma_start(rhs_x[:], (wih1 if layer == 1 else wih2)[:])
            whh_l = whh1 if layer == 1 else whh2
            bias_l, nkx = (bias1 if layer == 1 else bias2), 4
        whh_sb = wpool.tile([2, 128, G], BF16, tag="whh")
        nc.sync.dma_start(whh_sb[:], whh_l[:])
        if bias_l is not None:
            bias_sb = wpool.tile([1, G], BF16, tag="bias")
            nc.sync.dma_start(bias_sb[:], bias_l[:])

        # init state
        c_prev = cellp.tile([B_loc, H], BF16, tag="c")
        nc.vector.memset(c_prev[:], 0.0)
        hT = []
        for k in range(2):
            st = hTp.tile([128, B_loc], BF16, tag=f"hTsb{k}")
            nc.vector.memset(st[:], 0.0)
            hT.append(st)

        for ch in range(NCH):
            # stage this chunk's x inputs
            if layer == 0:
                trow = xin.tile([1, CHUNK * B_loc], I32, tag="trow")
                nc.sync.dma_start(trow[:], textT[:, ch * CHUNK * B_loc:(ch + 1) * CHUNK * B_loc])
                tb = xin.tile([128, CHUNK * B_loc], I32, tag="tbcast")
                nc.gpsimd.partition_broadcast(tb[:], trow[:])
                xch = []
                for k in range(2):
                    oh = xin.tile([128, CHUNK * B_loc], BF16, tag=f"oh{k}")
                    nc.vector.tensor_tensor(oh[:], iota2[k][:], tb[:],
                                            op=mybir.AluOpType.is_equal)
                    xch.append(oh)
            else:
                xch = []
                for d in range(2):
                    for k in range(2):
                        xt = xin.tile([128, CHUNK * B_loc], BF16, tag=f"x{d}{k}")
                        nc.sync.dma_start(
                            xt[:], recv[d, k, :, ch * CHUNK * B_loc:(ch + 1) * CHUNK * B_loc])
                        xch.append(xt)

            for tau in range(CHUNK):
                t = ch * CHUNK + tau
                sl = slice(tau * B_loc, (tau + 1) * B_loc)
                gates = psg.tile([B_loc, G], F32, tag="gates")
                for n in range(2):
                    nsl = slice(n * 512, (n + 1) * 512)
                    first = True
                    for kx in range(len(xch)):
                        nc.tensor.matmul(gates[:, nsl], xch[kx][:, sl],
                                         rhs_x[kx, :, nsl], start=first, stop=False)
                        first = False
                    for kh in range(2):
                        nc.tensor.matmul(gates[:, nsl], hT[kh][:], whh_sb[kh, :, nsl],
                                         start=False, stop=(layer == 0))
                    if layer > 0:
                        nc.tensor.matmul(gates[:, nsl], ones1[:], bias_sb[:, nsl],
                                         start=False, stop=True)
                h, c_prev = _cell(nc, pools, gates, c_prev, B_loc)
                hT = _transpose_h(nc, pools, h, ident, B_loc)
                for k in range(2):
                    nc.sync.dma_start(send[k, :, t * B_loc:(t + 1) * B_loc], hT[k][:])

        # exchange archives (AllGather pairs: slot0=fwd cores, slot1=rev cores)
        nc.gpsimd.collective_compute(
            "AllGather", mybir.AluOpType.bypass, replica_groups=GROUPS,
            ins=[send[:]], outs=[recv[:]])

    # ---------------- decoder ----------------
    tbld_sb = wpool.tile([2, 128, G], BF16, tag="rhsx")  # reuse slot
    nc.sync.dma_start(tbld_sb[:], tbld[:])
    dwih0_sb = wpool.tile([4, 128, G], BF16, tag="rhsx4")
    nc.sync.dma_start(dwih0_sb[:], dwih0f[:])
    dwhh_sb = wpool.tile([3, 2, 128, G], BF16, tag="dwhh")
    nc.sync.dma_start(dwhh_sb[:], dwhh[:])
    dwih_sb = wpool.tile([2, 2, 128, G], BF16, tag="dwih")
    nc.sync.dma_start(dwih_sb[:], dwih[:])
    dbias_sb = wpool.tile([2, 1, G], BF16, tag="dbias")
    nc.sync.dma_start(dbias_sb[:], dbias[:])
    dbias0_sb = wpool.tile([1, G], BF16, tag="bias")
    nc.sync.dma_start(dbias0_sb[:], dbias0[:])
    projwT_sb = wpool.tile([2, 128, V], BF16, tag="projwT")
    nc.sync.dma_start(projwT_sb[:], projwT[:])

    # dec_in0T: [fwd t=0 ; rev t=T-1] from final-layer recv
    din0 = []
    for d in range(2):
        tcol = 0 if d == 0 else T - 1
        for k in range(2):
            xt = wpool.tile([128, B_loc], BF16, tag=f"din{d}{k}")
            nc.sync.dma_start(xt[:], recv[d, k, :, tcol * B_loc:(tcol + 1) * B_loc])
            din0.append(xt)

    logarch = wpool.tile([2, 128, DEC * B_loc], BF16, tag="logarch")

    hTs, cs = [], []
    for l in range(3):
        row = []
        for k in range(2):
            st = hTp.tile([128, B_loc], BF16, tag=f"dhT{l}{k}")
            nc.vector.memset(st[:], 0.0)
            row.append(st)
        hTs.append(row)
        c0 = cellp.tile([B_loc, H], BF16, tag=f"dc{l}")
        nc.vector.memset(c0[:], 0.0)
        cs.append(c0)
    oh_fb = None

    for s in range(DEC):
        for l in range(3):
            gates = psg.tile([B_loc, G], F32, tag="gates")
            for n in range(2):
                nsl = slice(n * 512, (n + 1) * 512)
                if l == 0:
                    if s == 0:
                        for kx in range(4):
                            nc.tensor.matmul(gates[:, nsl], din0[kx][:],
                                             dwih0_sb[kx, :, nsl], start=(kx == 0), stop=False)
                        nc.tensor.matmul(gates[:, nsl], ones1[:], dbias0_sb[:, nsl],
                                         start=False, stop=False)
                    else:
                        for kx in range(2):
                            nc.tensor.matmul(gates[:, nsl], oh_fb[kx][:],
                                             tbld_sb[kx, :, nsl], start=(kx == 0), stop=False)
                else:
                    for kx in range(2):
                        nc.tensor.matmul(gates[:, nsl], hTs[l - 1][kx][:],
                                         dwih_sb[l - 1, kx, :, nsl], start=(kx == 0), stop=False)
                    nc.tensor.matmul(gates[:, nsl], ones1[:], dbias_sb[l - 1, :, nsl],
                                     start=False, stop=False)
                for kh in range(2):
                    nc.tensor.matmul(gates[:, nsl], hTs[l][kh][:],
                                     dwhh_sb[l, kh, :, nsl], start=False, stop=(kh == 1))
            h, c_new = _cell(nc, pools, gates, cs[l], B_loc)
            cs[l] = c_new
            hTs[l] = _transpose_h(nc, pools, h, ident, B_loc, tag=f"dhT{l}")

        # projection (transposed): logitsT (2 x (128, B_loc))
        lt_sb = []
        for m in range(2):
            lps = psT.tile([128, B_loc], F32, tag=f"lps{m}")
            for k in range(2):
                nc.tensor.matmul(lps[:], projwT_sb[k, :, m * 128:(m + 1) * 128],
                                 hTs[2][k][:], start=(k == 0), stop=(k == 1))
            dst = logarch[m, :, s * B_loc:(s + 1) * B_loc]
            nc.scalar.activation(dst, lps[:], AF.Identity, bias=projb_sb[:, m:m + 1])
            lt_sb.append(dst)

        # argmax -> one-hot feedback
        m01 = cellp.tile([128, B_loc], BF16, tag="m01")
        nc.vector.tensor_max(m01[:], lt_sb[0], lt_sb[1])
        pmax = cellp.tile([128, B_loc], BF16, tag="pmax")
        nc.gpsimd.partition_all_reduce(pmax[:], m01[:], channels=128,
                                       reduce_op=bass_isa.ReduceOp.max)
        oh_fb = []
        for m in range(2):
            oh = cellp.tile([128, B_loc], BF16, tag=f"ohfb{m}")
            nc.vector.tensor_tensor(oh[:], lt_sb[m], pmax[:],
                                    op=mybir.AluOpType.is_equal)
            oh_fb.append(oh)

    nc.sync.dma_start(logitsT_out[:], logarch[:])


# ---------------- host side ----------------

def _prep_inputs(inputs, B_loc, T, DEC):
    """Build per-core in_maps (8 cores)."""
    f32 = np.float32
    text = np.asarray(inputs["text_bytes"]).astype(np.int64)
    emb = np.asarray(inputs["emb"], f32)
    ewih0 = np.asarray(inputs["enc_w_ih_l0"], f32)
    ewhh0 = np.asarray(inputs["enc_w_hh_l0"], f32)
    eb0 = np.asarray(inputs["enc_b_l0"], f32)
    ewihr = np.asarray(inputs["enc_w_ih_rest"], f32)
    ewhhr = np.asarray(inputs["enc_w_hh_rest"], f32)
    ebr = np.asarray(inputs["enc_b_rest"], f32)
    dwih0 = np.asarray(inputs["dec_w_ih_l0"], f32)
    dwihr = np.asarray(inputs["dec_w_ih_rest"], f32)
    dwhh_ = np.asarray(inputs["dec_w_hh"], f32)
    db = np.asarray(inputs["dec_b"], f32)
    pw = np.asarray(inputs["proj_w"], f32)
    pb = np.asarray(inputs["proj_b"], f32)

    def bt(a):
        return np.ascontiguousarray(a.astype(nbf16))

    def kchunks(m):  # (K, G) -> (K//128, 128, G)
        return m.reshape(m.shape[0] // 128, 128, m.shape[1])

    common = {}
    # decoder tables (shared by all cores)
    common["tbld"] = bt(kchunks(_reord(emb @ dwih0[:, :H].T + db[0], 1)))
    common["dwih0f"] = bt(kchunks(_reord(dwih0.T, 1)))
    common["dbias0"] = bt(_reord(db[0], 0)[None, :])
    common["dwih"] = bt(np.stack([kchunks(_reord(dwihr[i].T, 1)) for i in range(2)]))
    common["dwhh"] = bt(np.stack([kchunks(_reord(dwhh_[i].T, 1)) for i in range(3)]))
    common["dbias"] = bt(np.stack([_reord(db[1 + i], 0)[None, :] for i in range(2)]))
    common["projwT"] = bt(pw.T.reshape(2, 128, V))
    common["projb"] = np.ascontiguousarray(pb.reshape(2, 128).T, dtype=f32)

    per_dir = {}
    for d in range(2):
        pd = {}
        pd["tbl0"] = bt(kchunks(_reord(emb @ ewih0[d].T + eb0[d], 1)))
        pd["whh0"] = bt(kchunks(_reord(ewhh0[d].T, 1)))
        for li, nm in ((0, "1"), (1, "2")):
            pd["wih" + nm] = bt(kchunks(_reord(ewihr[li, d].T, 1)))
            pd["whh" + nm] = bt(kchunks(_reord(ewhhr[li, d].T, 1)))
            pd["bias" + nm] = bt(_reord(ebr[li, d], 0)[None, :])
        per_dir[d] = pd

    in_maps = []
    for c in range(8):
        d, q = c // 4, c % 4
        rows = text[q * B_loc:(q + 1) * B_loc, :T]
        if d == 1:
            rows = rows[:, ::-1]
        m = dict(common)
        m.update(per_dir[d])
        m["textT"] = np.ascontiguousarray(rows.T.reshape(1, T * B_loc), dtype=np.int32)
        in_maps.append(m)
    return in_maps


_PROG_CACHE = {}


def kernel(**inputs):
    from concourse import bass_utils
    text = np.asarray(inputs["text_bytes"])
    B, T = text.shape
    DEC = int(inputs["max_nefs_len"])
    B_loc = B // 4
    key = (B_loc, T, DEC)
    if key not in _PROG_CACHE:
        _PROG_CACHE[key] = build_program(B_loc, T, DEC, CHUNK=32)
    nc = _PROG_CACHE[key]
    in_maps = _prep_inputs(inputs, B_loc, T, DEC)
    res = bass_utils.run_bass_kernel_spmd(nc, in_maps, list(range(8)))
    out = np.zeros((B, DEC, V), np.float32)
    for q in range(4):
        lt = res.results[q]["logitsT"].astype(np.float32)  # (2,128,DEC*B_loc)
        lt = lt.reshape(2, 128, DEC, B_loc)  # v-chunk, v-part, t, b
        out[q * B_loc:(q + 1) * B_loc] = lt.transpose(3, 2, 0, 1).reshape(B_loc, DEC, V)
    return out


# revision 13
# speedup vs baseline: 1.2504x; 1.2504x over previous
"""Trainium2 Bass kernel for DirectG2PNEFS: 3-layer biLSTM encoder (B=256,T=512,H=256)
+ 100-step argmax-feedback LSTM decoder.

Sharding: 8 cores = 2 directions x 4 batch-quarters (64 rows each), one SPMD
program, all divergence data-driven. Matmuls bf16 (validated vs f32: 0 argmax
flips, ~4e-3 rel err), cell math on f32 PSUM accum, state bf16.

Per-core encoder scan (batch on partitions, hidden transposed each step on PE):
  gates(64,1024)fp32 = onehot/x_T-stationary @ W_ih-moving + h_T @ W_hh-moving [+ bias]
Layer exchange: per-pair AllGather of transposed-h archives (fwd slot 0, rev slot 1).
Decoder: duplicated per pair, argmax via partition_all_reduce(max) + is_equal onehot.
"""
from contextlib import ExitStack

import numpy as np
import ml_dtypes

import concourse.bass as bass
from concourse import bacc
import concourse.mybir as mybir
import concourse.tile as tile
from concourse import bass_isa
from concourse import library_config

BF16 = mybir.dt.bfloat16
F32 = mybir.dt.float32
I32 = mybir.dt.int32
AF = mybir.ActivationFunctionType
OP = mybir.AluOpType
nbf16 = ml_dtypes.bfloat16

H = 256
V = 256
G = 4 * H
# gate reorder: torch order i,f,g,o -> ours i,f,o,g (sigmoid group contiguous)
GATE_PERM = np.concatenate([np.arange(0, H), np.arange(H, 2 * H),
                            np.arange(3 * H, 4 * H), np.arange(2 * H, 3 * H)])


def _reord(w, axis):
    return np.take(w, GATE_PERM, axis=axis)


def build_program(B_loc, T, DEC, CHUNK):
    assert T % CHUNK == 0
    NCH = T // CHUNK
    CB = CHUNK * B_loc
    nc = bacc.Bacc("TRN2", target_bir_lowering=False, debug=False, num_devices=8)
    def dp(name, shape, dtype, isOutput=False):
        return nc.declare_dram_parameter(name, shape, dtype, isOutput=isOutput)

    textT = dp("textT", [1, T * B_loc], I32)
    enc_w = {}
    enc_w[0] = dict(
        rx=[dp("tbl0_%d" % k, [128, G], BF16) for k in range(2)],
        whh=[dp("whh0_%d" % k, [128, G], BF16) for k in range(2)], bias=None)
    for l in (1, 2):
        enc_w[l] = dict(
            rx=[dp("wih%d_%d" % (l, k), [128, G], BF16) for k in range(4)],
            whh=[dp("whh%d_%d" % (l, k), [128, G], BF16) for k in range(2)],
            bias=dp("bias%d" % l, [1, G], BF16))
    tbld = [dp("tbld_%d" % k, [128, G], BF16) for k in range(2)]
    dwih0 = [dp("dwih0_%d" % k, [128, G], BF16) for k in range(4)]
    dbias0 = dp("dbias0", [1, G], BF16)
    dwih = [[dp("dwih%d_%d" % (l, k), [128, G], BF16) for k in range(2)] for l in (1, 2)]
    dwhh = [[dp("dwhh%d_%d" % (l, k), [128, G], BF16) for k in range(2)] for l in range(3)]
    dbias = [dp("dbias%d" % l, [1, G], BF16) for l in (1, 2)]
    projwT = [dp("projwT_%d" % k, [128, V], BF16) for k in range(2)]
    projb = dp("projb", [128, 2], F32)
    ident_dram = dp("ident", [128, 128], BF16)
    iota_dram = [dp("iota_%d" % k, [128, CHUNK * B_loc], I32) for k in range(2)]
    logitsT_out = dp("logitsT", [2, 128, DEC * B_loc], BF16, isOutput=True)

    send = nc.dram_tensor("send", [2, 128, T * B_loc], BF16)
    recv = nc.dram_tensor("recv", [2, 2, 128, T * B_loc], BF16)
    GROUPS = [[0, 1], [2, 3], [4, 5], [6, 7]]

    with tile.TileContext(nc) as tc, ExitStack() as ctx:
        consts = ctx.enter_context(tc.tile_pool(name="consts", bufs=1))
        wpool = ctx.enter_context(tc.tile_pool(name="weights", bufs=1))
        xin = ctx.enter_context(tc.tile_pool(name="xin", bufs=2))
        cellp = ctx.enter_context(tc.tile_pool(name="cellp", bufs=2))
        hTp = ctx.enter_context(tc.tile_pool(name="hTp", bufs=2))
        psg = ctx.enter_context(tc.tile_pool(name="psg", bufs=2, space="PSUM"))
        psT = ctx.enter_context(tc.tile_pool(name="psT", bufs=1, space="PSUM"))
        pslg = ctx.enter_context(tc.tile_pool(name="pslg", bufs=1, space="PSUM"))

        ident = consts.tile([128, 128], BF16)
        nc.sync.dma_start(ident[:], ident_dram[:])
        ones1 = consts.tile([1, B_loc], BF16)
        nc.vector.memset(ones1[:], 1.0)
        iota2 = []
        for k in range(2):
            it = consts.tile([128, CB], I32, tag="iota%d" % k)
            nc.sync.dma_start(it[:], iota_dram[k][:])
            iota2.append(it)
        projb_sb = consts.tile([128, 2], F32)
        nc.sync.dma_start(projb_sb[:], projb[:])

        def load_w(drams, tag):
            tiles = []
            for i, d in enumerate(drams):
                t = wpool.tile(list(d.shape), d.dtype, tag="%s%d" % (tag, i), name="w_%s%d" % (tag, i))
                nc.sync.dma_start(t[:], d[:])
                tiles.append(t)
            return tiles

        def cell(gates_ps, c_prev, tagp=""):
            sig = cellp.tile([B_loc, 3 * H], BF16, tag="sig")
            nc.scalar.activation(sig[:], gates_ps[:, 0:3 * H], AF.Sigmoid)
            gt = cellp.tile([B_loc, H], BF16, tag="gt")
            nc.scalar.activation(gt[:], gates_ps[:, 3 * H:4 * H], AF.Tanh)
            w = cellp.tile([B_loc, H], BF16, tag="w")
            nc.vector.tensor_mul(w[:], sig[:, 0:H], gt[:])
            t1 = cellp.tile([B_loc, H], BF16, tag="t1")
            nc.vector.tensor_mul(t1[:], sig[:, H:2 * H], c_prev[:])
            c_new = cellp.tile([B_loc, H], BF16, tag="c" + tagp)
            nc.vector.tensor_add(c_new[:], t1[:], w[:])
            tc_t = cellp.tile([B_loc, H], BF16, tag="tc")
            nc.scalar.activation(tc_t[:], c_new[:], AF.Tanh)
            h = cellp.tile([B_loc, H], BF16, tag="h")
            nc.vector.tensor_mul(h[:], sig[:, 2 * H:3 * H], tc_t[:])
            return h, c_new

        def transpose_h(h, sbtag):
            outs = []
            for k in range(2):
                pt = psT.tile([128, B_loc], BF16, tag="hTps%d" % k)
                nc.tensor.transpose(pt[:], h[:, k * 128:(k + 1) * 128],
                                    ident[:B_loc, :B_loc])
                st = hTp.tile([128, B_loc], BF16, tag="%s%d" % (sbtag, k))
                nc.vector.tensor_copy(st[:], pt[:])
                outs.append(st)
            return outs

        # ---------------- encoder ----------------
        for layer in range(3):
            lw = enc_w[layer]
            rx = load_w(lw["rx"], "rx")
            whh_sb = load_w(lw["whh"], "whh")
            bias_sb = load_w([lw["bias"]], "bias")[0] if lw["bias"] is not None else None

            c_prev = cellp.tile([B_loc, H], BF16, tag="c")
            nc.vector.memset(c_prev[:], 0.0)
            hT = []
            for k in range(2):
                st = hTp.tile([128, B_loc], BF16, tag="hT%d" % k)
                nc.vector.memset(st[:], 0.0)
                hT.append(st)

            def stage(chk):
                csl = slice(chk * CB, (chk + 1) * CB)
                if layer == 0:
                    trow = xin.tile([1, CB], I32, tag="trow", name="trow")
                    nc.sync.dma_start(trow[:], textT[:, csl])
                    tb = xin.tile([128, CB], I32, tag="tbcast", name="tb")
                    nc.gpsimd.partition_broadcast(tb[:], trow[:])
                    xch_ = []
                    for k in range(2):
                        oh = xin.tile([128, CB], BF16, tag="oh%d" % k, name="oh")
                        nc.vector.tensor_tensor(oh[:], iota2[k][:], tb[:], op=OP.is_equal)
                        xch_.append(oh)
                    return xch_
                xch_ = []
                for d in range(2):
                    for k in range(2):
                        xt = xin.tile([128, CB], BF16, tag="x%d%d" % (d, k), name="xt")
                        nc.sync.dma_start(xt[:], recv[d, k, :, csl])
                        xch_.append(xt)
                return xch_

            def issue_xg(xch_, tau_):
                sl = slice(tau_ * B_loc, (tau_ + 1) * B_loc)
                g = psg.tile([B_loc, G], F32, tag="gates", name="gates")
                for n in range(2):
                    nsl = slice(n * 512, (n + 1) * 512)
                    for kx in range(len(xch_)):
                        nc.tensor.matmul(g[:, nsl], xch_[kx][:, sl],
                                         rx[kx][:, nsl], start=(kx == 0), stop=False)
                    if bias_sb is not None:
                        nc.tensor.matmul(g[:, nsl], ones1[:], bias_sb[:, nsl],
                                         start=False, stop=False)
                return g

            xch = stage(0)
            gates = None
            for chk in range(NCH):
                xch_next = stage(chk + 1) if chk + 1 < NCH else None
                for tau in range(CHUNK):
                    t = chk * CHUNK + tau
                    if gates is None:
                        gates = issue_xg(xch, tau)
                    for n in range(2):
                        nsl = slice(n * 512, (n + 1) * 512)
                        for kh in range(2):
                            nc.tensor.matmul(gates[:, nsl], hT[kh][:], whh_sb[kh][:, nsl],
                                             start=False, stop=(kh == 1))
                    h, c_prev = cell(gates, c_prev)
                    if tau + 1 < CHUNK:
                        gates = issue_xg(xch, tau + 1)
                    elif xch_next is not None:
                        gates = issue_xg(xch_next, 0)
                    else:
                        gates = None
                    hT = transpose_h(h, "hT")
                    for k in range(2):
                        nc.sync.dma_start(send[k, :, t * B_loc:(t + 1) * B_loc], hT[k][:])
                xch = xch_next

            nc.gpsimd.collective_compute(
                "AllGather", OP.bypass, replica_groups=GROUPS,
                ins=[send[:]], outs=[recv[:]])

        # ---------------- decoder ----------------
        tbld_sb = load_w(tbld, "rx")  # reuse encoder weight slots
        dwih0_sb = load_w(dwih0, "dw0")
        dwih_sb = [load_w(dwih[i], "dwi%d" % i) for i in range(2)]
        dwhh_sb = [load_w(dwhh[l], "dwh%d" % l) for l in range(3)]
        dbias_sb = load_w(dbias, "dbias")
        dbias0_sb = load_w([dbias0], "bias")[0]
        projwT_sb = load_w(projwT, "projwT")

        din0 = []
        for d in range(2):
            tcol = 0 if d == 0 else T - 1
            for k in range(2):
                xt = wpool.tile([128, B_loc], BF16, tag="din%d%d" % (d, k))
                nc.sync.dma_start(xt[:], recv[d, k, :, tcol * B_loc:(tcol + 1) * B_loc])
                din0.append(xt)

        logarch = [wpool.tile([128, DEC * B_loc], BF16, tag="logarch%d" % m,
                               name="logarch%d" % m) for m in range(2)]

        hTs, cs = [], []
        for l in range(3):
            row = []
            for k in range(2):
                st = hTp.tile([128, B_loc], BF16, tag="dhT%d%d" % (l, k))
                nc.vector.memset(st[:], 0.0)
                row.append(st)
            hTs.append(row)
            c0 = cellp.tile([B_loc, H], BF16, tag="cd%d" % l)
            nc.vector.memset(c0[:], 0.0)
            cs.append(c0)
        oh_fb = None

        for s in range(DEC):
            for l in range(3):
                gates = psg.tile([B_loc, G], F32, tag="gates")
                for n in range(2):
                    nsl = slice(n * 512, (n + 1) * 512)
                    if l == 0:
                        if s == 0:
                            for kx in range(4):
                                nc.tensor.matmul(gates[:, nsl], din0[kx][:],
                                                 dwih0_sb[kx][:, nsl],
                                                 start=(kx == 0), stop=False)
                            nc.tensor.matmul(gates[:, nsl], ones1[:], dbias0_sb[:, nsl],
                                             start=False, stop=False)
                        else:
                            for kx in range(2):
                                nc.tensor.matmul(gates[:, nsl], oh_fb[kx][:],
                                                 tbld_sb[kx][:, nsl],
                                                 start=(kx == 0), stop=False)
                    else:
                        for kx in range(2):
                            nc.tensor.matmul(gates[:, nsl], hTs[l - 1][kx][:],
                                             dwih_sb[l - 1][kx][:, nsl],
                                             start=(kx == 0), stop=False)
                        nc.tensor.matmul(gates[:, nsl], ones1[:],
                                         dbias_sb[l - 1][:, nsl], start=False, stop=False)
                    for kh in range(2):
                        nc.tensor.matmul(gates[:, nsl], hTs[l][kh][:],
                                         dwhh_sb[l][kh][:, nsl],
                                         start=False, stop=(kh == 1))
                h, c_new = cell(gates, cs[l], tagp="d%d" % l)
                cs[l] = c_new
                hTs[l] = transpose_h(h, "dhT%d" % l)

            lps = pslg.tile([128, 2 * B_loc], F32, tag="lps")
            lt_sb = []
            for m in range(2):
                msl = slice(m * B_loc, (m + 1) * B_loc)
                for k in range(2):
                    nc.tensor.matmul(lps[:, msl], projwT_sb[k][:, m * 128:(m + 1) * 128],
                                     hTs[2][k][:], start=(k == 0), stop=(k == 1))
                dst = logarch[m][:, s * B_loc:(s + 1) * B_loc]
                nc.scalar.activation(dst, lps[:, msl], AF.Identity,
                                     bias=projb_sb[:, m:m + 1])
                lt_sb.append(dst)

            m01 = cellp.tile([128, B_loc], BF16, tag="m01")
            nc.vector.tensor_max(m01[:], lt_sb[0], lt_sb[1])
            pmax = cellp.tile([128, B_loc], BF16, tag="pmax")
            nc.gpsimd.partition_all_reduce(pmax[:], m01[:], channels=128,
                                           reduce_op=bass_isa.ReduceOp.max)
            oh_fb = []
            for m in range(2):
                oh = cellp.tile([128, B_loc], BF16, tag="ohfb%d" % m)
                nc.vector.tensor_tensor(oh[:], lt_sb[m], pmax[:], op=OP.is_equal)
                oh_fb.append(oh)

        for m in range(2):
            nc.sync.dma_start(logitsT_out[m], logarch[m][:])

    nc.compile()
    return nc


# ---------------- host side ----------------

def _prep_inputs(inputs, B_loc, T, DEC, CHUNK=32):
    f32 = np.float32
    text = np.asarray(inputs["text_bytes"]).astype(np.int64)
    emb = np.asarray(inputs["emb"], f32)
    ewih0 = np.asarray(inputs["enc_w_ih_l0"], f32)
    ewhh0 = np.asarray(inputs["enc_w_hh_l0"], f32)
    eb0 = np.asarray(inputs["enc_b_l0"], f32)
    ewihr = np.asarray(inputs["enc_w_ih_rest"], f32)
    ewhhr = np.asarray(inputs["enc_w_hh_rest"], f32)
    ebr = np.asarray(inputs["enc_b_rest"], f32)
    dwih0 = np.asarray(inputs["dec_w_ih_l0"], f32)
    dwihr = np.asarray(inputs["dec_w_ih_rest"], f32)
    dwhh_ = np.asarray(inputs["dec_w_hh"], f32)
    db = np.asarray(inputs["dec_b"], f32)
    pw = np.asarray(inputs["proj_w"], f32)
    pb = np.asarray(inputs["proj_b"], f32)

    def bt(a):
        return np.ascontiguousarray(a.astype(nbf16))

    def kch(m, name, out):  # (K, G) -> tensors name_k (128, G)
        for k in range(m.shape[0] // 128):
            out[name + "_%d" % k] = bt(m[k * 128:(k + 1) * 128])

    common = {}
    kch(_reord(emb @ dwih0[:, :H].T + db[0], 1), "tbld", common)
    kch(_reord(dwih0.T, 1), "dwih0", common)
    common["dbias0"] = bt(_reord(db[0], 0)[None, :])
    for i in range(2):
        kch(_reord(dwihr[i].T, 1), "dwih%d" % (i + 1), common)
        common["dbias%d" % (i + 1)] = bt(_reord(db[1 + i], 0)[None, :])
    for l in range(3):
        kch(_reord(dwhh_[l].T, 1), "dwhh%d" % l, common)
    kch(pw.T, "projwT", common)
    common["projb"] = np.ascontiguousarray(pb.reshape(2, 128).T, dtype=f32)
    common["ident"] = np.eye(128, dtype=nbf16)
    for k in range(2):
        common["iota_%d" % k] = np.ascontiguousarray(
            np.broadcast_to(np.arange(128, dtype=np.int32)[:, None] + 128 * k,
                            (128, CHUNK * B_loc)))

    per_dir = []
    for d in range(2):
        pd = {}
        kch(_reord(emb @ ewih0[d].T + eb0[d], 1), "tbl0", pd)
        kch(_reord(ewhh0[d].T, 1), "whh0", pd)
        for li in range(2):
            kch(_reord(ewihr[li, d].T, 1), "wih%d" % (li + 1), pd)
            kch(_reord(ewhhr[li, d].T, 1), "whh%d" % (li + 1), pd)
            pd["bias%d" % (li + 1)] = bt(_reord(ebr[li, d], 0)[None, :])
        per_dir.append(pd)

    in_maps = []
    for c in range(8):
        d, q = c % 2, c // 2
        rows = text[q * B_loc:(q + 1) * B_loc, :T]
        if d == 1:
            rows = rows[:, ::-1]
        m = dict(common)
        m.update(per_dir[d])
        m["textT"] = np.ascontiguousarray(rows.T.reshape(1, T * B_loc), dtype=np.int32)
        in_maps.append(m)
    return in_maps


_PROG_CACHE = {}


def kernel(**inputs):
    from concourse import bass_utils
    text = np.asarray(inputs["text_bytes"])
    B, T = text.shape
    DEC = int(inputs["max_nefs_len"])
    B_loc = B // 4
    key = (B_loc, T, DEC)
    if key not in _PROG_CACHE:
        _PROG_CACHE[key] = build_program(B_loc, T, DEC, CHUNK=32)
    nc = _PROG_CACHE[key]
    in_maps = _prep_inputs(inputs, B_loc, T, DEC, CHUNK=32)
    res = bass_utils.run_bass_kernel_spmd(nc, in_maps, list(range(8)))
    out = np.zeros((B, DEC, V), np.float32)
    for q in range(4):
        lt = res.results[2 * q]["logitsT"].astype(np.float32)  # (2,128,DEC*B_loc)
        lt = lt.reshape(2, 128, DEC, B_loc)
        out[q * B_loc:(q + 1) * B_loc] = lt.transpose(3, 2, 0, 1).reshape(B_loc, DEC, V)
    return out
